# revision 22
# baseline (speedup 1.0000x reference)
"""Trainium2 Bass kernel for nn_Attention_84473416778449.

Reference computation (B=2, S=2048, D=1024, H=16, HD=64, fp32):
    q/k/v = x @ w{q,k,v}.T ; RoPE(q, k) ; causal softmax attention ; out @ wo.T

Sharding: 8 cores = (batch 2) x (head-group 4). Each core computes 4 heads of
one batch end-to-end and a partial output projection over its 256 channels;
the host sums the 4 partials per batch.

Key design points (v3):
  - All device inputs fp16 (host pre-converts); output fp16, summed on host.
  - RoPE without a PE matmul: host permutes q/k output channels so RoPE
    pairs (d, d+32) sit adjacent; rotate_half becomes a DVE stream_shuffle
    (adjacent-pair swap within 32-blocks), signs folded into the sin table.
  - Scores for the two heads of a pair run concurrently on the PE via row
    tiling (tile_position (0,0) / (64,0)), writing halves of one
    [128, 1024] PSUM strip consumed by a single batched exp per kt.
  - Wavefront + fillers: projections of later blocks and output projections
    of earlier blocks are interleaved between attention matmul groups so no
    engine idles long; filler lists are balanced per phase.
  - Pair-boundary decoupling: at each pair end only the two PSUM-freeing
    copies run immediately (ACT: PV rows, DVE: denominator row); the
    reciprocal/broadcast/normalize-multiplies are deferred into the next
    pair's iterations.
  - Final block output projection is hp-split: the attnT[0] half runs as
    fillers during the last pair; only the attnT[1] matmul + add + DMA
    remain in the tail.
  - PSUM (8 banks): proj 2 + score strips 4 + PV accumulator pair 2.
"""
import sys

if "/opt/trn_rl_repo" not in sys.path:
    sys.path.insert(0, "/opt/trn_rl_repo")

import numpy as np

import concourse.bass as bass
import concourse.mybir as mybir
import concourse.tile as tile
from concourse import bacc
from concourse.bass_utils import run_bass_kernel_spmd

B, S, D, H, HD = 2, 2048, 1024, 16, 64
NCORES = 8
GROUPS = 4            # head groups
GH = H // GROUPS      # heads per group = 4
GC = GH * HD          # channels per group = 256
KT = D // 128         # 8 k-tiles over D
ST = S // 128         # 16 s-tiles
QB = 4                # sq blocks of 512
QW = S // QB          # 512
VW = GH * (HD + 1)    # 260: v tile payload columns
VPAD = 3 * (HD + 1) + 128   # 323: pad so stationary window fits for h=3

f32 = mybir.dt.float32
MMDT = mybir.dt.float16   # matmul-operand dtype
Exp = mybir.ActivationFunctionType.Exp
Copy = mybir.ActivationFunctionType.Copy

# stream_shuffle mask: swap adjacent pairs within each 32-partition quadrant
SWAP_MASK = [j ^ 1 for j in range(32)]

# stride-0 partition APs are rejected by the DVE ("partition dimension must
# have nonzero step") — go through the gpsimd partition_broadcast
USE_BCAST_AP = False

_cache = {}


def _build():
    nc = bacc.Bacc("TRN2", num_devices=NCORES)

    # x and q/k/v weights arrive pre-arranged in the exact SBUF layout so
    # every load is a contiguous full-rate DMA (8KB / 4KB partition lines)
    xS = nc.dram_tensor("xS", [128, QB * KT * QW], MMDT,
                        kind="ExternalInput").ap()
    wqS = nc.dram_tensor("wqS", [128, KT * GC], MMDT,
                         kind="ExternalInput").ap()
    wkS = nc.dram_tensor("wkS", [128, KT * GC], MMDT,
                         kind="ExternalInput").ap()
    wvS = nc.dram_tensor("wvS", [128, KT * GC], MMDT,
                         kind="ExternalInput").ap()
    woT = nc.dram_tensor("woT", [GC, D], MMDT, kind="ExternalInput").ap()
    cs2 = nc.dram_tensor("cs2", [128, S], MMDT, kind="ExternalInput").ap()
    sn2 = nc.dram_tensor("sn2", [128, S], MMDT, kind="ExternalInput").ap()
    out = nc.dram_tensor("out", [S, D], MMDT, kind="ExternalOutput").ap()

    with tile.TileContext(nc) as tc:
        with tc.tile_pool(name="persist", bufs=1) as pp, \
             tc.tile_pool(name="rope", bufs=3) as rp, \
             tc.tile_pool(name="probs", bufs=4) as wp, \
             tc.tile_pool(name="outsb", bufs=3) as op_, \
             tc.tile_pool(name="ob0s", bufs=8) as o0p, \
             tc.tile_pool(name="norm", bufs=2) as sp:

            # ---- first-needed loads go out before anything else ------------
            xTb = [pp.tile([128, KT * QW], MMDT, tag=f"xTb{cb}",
                           name=f"xTb{cb}") for cb in range(QB)]

            def xTr(kt, cb):
                return xTb[cb][:, kt * QW:(kt + 1) * QW]

            def load_x_block(cb, chunks=1, eng=None):
                eng = eng or nc.gpsimd
                w = KT * QW // chunks
                for j in range(chunks):
                    eng.dma_start(
                        xTb[cb][:, j * w:(j + 1) * w],
                        xS[:, cb * KT * QW + j * w:cb * KT * QW + (j + 1) * w])

            def load_wT(srcS, eng):
                t = pp.tile([128, KT * GC], MMDT, tag=f"w{srcS.tensor.name}",
                            name=f"w{srcS.tensor.name}")
                eng.dma_start(t[:], srcS[:])
                return t

            # spread the startup loads over the three DMA-capable queues;
            # x1 and wo are triggered later, gated behind compute sems, so
            # the first-needed loads monopolize early DMA bandwidth
            wq_s = load_wT(wqS, nc.sync)
            load_x_block(0, chunks=4)
            wk_s = load_wT(wkS, nc.scalar)
            cs_sb = pp.tile([128, S], MMDT, tag="cs")
            nc.sync.dma_start(cs_sb[:], cs2[:])
            sn_sb = pp.tile([128, S], MMDT, tag="sn")
            nc.sync.dma_start(sn_sb[:], sn2[:])
            wv_s = load_wT(wvS, nc.scalar)
            wo_s = []
            for kt in range(2):
                t = pp.tile([128, D], MMDT, tag=f"wo{kt}", name=f"wo{kt}")
                wo_s.append(t)

            qT = [[pp.tile([128, QW], MMDT, tag=f"qT{i}_{b}",
                           name=f"qT{i}_{b}") for b in range(QB)]
                  for i in range(2)]
            kTt = [[pp.tile([128, QW], MMDT, tag=f"kT{i}_{b}",
                            name=f"kT{i}_{b}") for b in range(QB)]
                   for i in range(2)]
            attnT = [[pp.tile([128, QW], MMDT, tag=f"aT{i}_{b}",
                              name=f"aT{i}_{b}") for b in range(QB)]
                     for i in range(2)]
            v_sb = [pp.tile([128, VPAD], MMDT, tag=f"v{i}",
                            name=f"v{i}") for i in range(ST)]
            # one-time v-tile framing: ones column per head (the PV
            # denominator lands in psum row 64; engine partition bases must
            # be 32-aligned so row 64 is staged via a 1-partition DVE copy)
            for st in range(ST):
                vhe = v_sb[st][:, 0:VW].rearrange("p (h e) -> p h e", e=HD + 1)
                nc.gpsimd.memset(vhe[:, :, HD:HD + 1], 1.0)
                nc.gpsimd.memset(v_sb[st][:, VW:VPAD], 0.0)

            with tc.tile_pool(name="psP", bufs=2, space="PSUM") as psP, \
                 tc.tile_pool(name="psS", bufs=2, space="PSUM") as psS, \
                 tc.tile_pool(name="psO", bufs=1, space="PSUM") as psO:

                def emit_qk_proj(w_src, dst, hp, sb):
                    cols = slice(sb * QW, (sb + 1) * QW)
                    pq = psP.tile([128, QW], f32, tag="proj",
                                  name=f"pq_{0 if dst is qT else 1}_{hp}_{sb}")
                    for kt in range(KT):
                        nc.tensor.matmul(
                            pq[:],
                            w_src[:, kt * GC + hp * 128:
                                  kt * GC + hp * 128 + 128],
                            xTr(kt, sb),
                            start=(kt == 0), stop=(kt == KT - 1))
                    # RoPE: dst = pqh*cos + swap(pqh*snH); snH is the
                    # pre-swapped sign-folded sin table (host-built); swap is
                    # the adjacent-pair partition shuffle (fp16 in/out).
                    # pq is staged to fp16 SBUF first (DVE copy, 2x mode):
                    # this frees the PSUM bank quickly and lets the DVE
                    # multiplies run in 2x packed mode.
                    pqh = rp.tile([128, QW], MMDT, tag="pqh")
                    nc.vector.tensor_copy(pqh[:], pq[:])
                    tcs = rp.tile([128, QW], MMDT, tag="tcs")
                    nc.vector.tensor_tensor(
                        out=tcs[:], in0=pqh[:], in1=cs_sb[:, cols],
                        op=mybir.AluOpType.mult)
                    tsp = rp.tile([128, QW], MMDT, tag="tsp")
                    nc.vector.tensor_tensor(
                        out=tsp[:], in0=pqh[:], in1=sn_sb[:, cols],
                        op=mybir.AluOpType.mult)
                    tsn = rp.tile([128, QW], MMDT, tag="tsn")
                    nc.vector.stream_shuffle(tsn[:], tsp[:], SWAP_MASK)
                    nc.gpsimd.tensor_tensor(
                        out=dst[hp][sb][:], in0=tcs[:], in1=tsn[:],
                        op=mybir.AluOpType.add)

                def emit_v(st):
                    pv = psP.tile([128, QW], f32, tag="proj",
                                  name=f"pv_{st}")
                    for kt in range(KT):
                        nc.tensor.matmul(
                            pv[:, 0:GC],
                            xTr(kt, st // 4)[:, (st % 4) * 128:
                                             (st % 4) * 128 + 128],
                            wv_s[:, kt * GC:(kt + 1) * GC],
                            start=(kt == 0), stop=(kt == KT - 1))
                    vhe = v_sb[st][:, 0:VW].rearrange(
                        "p (h e) -> p h e", e=HD + 1)
                    nc.vector.tensor_copy(
                        vhe[:, :, 0:HD],
                        pv[:, 0:GC].rearrange("p (h d) -> p h d", d=HD))

                def emit_out(st, db, on_act=False):
                    pc = psP.tile([128, QW], f32, tag="proj",
                                  name=f"pc_{st}_{db}")
                    for hp in range(2):
                        nc.tensor.matmul(
                            pc[:],
                            attnT[hp][st // 4][:, (st % 4) * 128:
                                               (st % 4) * 128 + 128],
                            wo_s[hp][:, db * QW:(db + 1) * QW],
                            start=(hp == 0), stop=(hp == 1))
                    ob = op_.tile([128, QW], MMDT, tag="outsb")
                    if on_act:
                        nc.scalar.activation(ob[:], pc[:], Copy)
                    else:
                        nc.vector.tensor_copy(ob[:], pc[:])
                    nc.sync.dma_start(
                        out[st * 128:(st + 1) * 128,
                            db * QW:(db + 1) * QW], ob[:])

                # hp-split output projection for the final block
                ob0s = {}

                def emit_out_hp0(st, db):
                    pc = psP.tile([128, QW], f32, tag="proj",
                                  name=f"pc0_{st}_{db}")
                    nc.tensor.matmul(
                        pc[:],
                        attnT[0][st // 4][:, (st % 4) * 128:
                                          (st % 4) * 128 + 128],
                        wo_s[0][:, db * QW:(db + 1) * QW],
                        start=True, stop=True)
                    ob0 = o0p.tile([128, QW], MMDT, tag="ob0",
                                   name=f"ob0_{st}_{db}")
                    nc.vector.tensor_copy(ob0[:], pc[:])
                    ob0s[(st, db)] = ob0

                def emit_out_hp1(st, db):
                    pc = psP.tile([128, QW], f32, tag="proj",
                                  name=f"pc1_{st}_{db}")
                    nc.tensor.matmul(
                        pc[:],
                        attnT[1][st // 4][:, (st % 4) * 128:
                                          (st % 4) * 128 + 128],
                        wo_s[1][:, db * QW:(db + 1) * QW],
                        start=True, stop=True)
                    ob = op_.tile([128, QW], MMDT, tag="outsb")
                    nc.vector.tensor_tensor(
                        out=ob[:], in0=pc[:], in1=ob0s[(st, db)][:],
                        op=mybir.AluOpType.add)
                    nc.sync.dma_start(
                        out[st * 128:(st + 1) * 128,
                            db * QW:(db + 1) * QW], ob[:])


                def emit_pv(po, hp, qb, kt, nsk, prts):
                    prt, c0, cw = prts.pop(kt)
                    for i, h in enumerate((2 * hp, 2 * hp + 1)):
                        nc.tensor.matmul(
                            po[:, i * QW + c0:(i + 1) * QW],
                            v_sb[kt][:, h * (HD + 1):h * (HD + 1) + 128],
                            prt[:, i * QW:i * QW + cw],
                            start=(kt == 0), stop=(kt == nsk - 1))

                def emit_attention(qb, fillers0=(), fillers1=()):
                    nsk = (qb + 1) * 4
                    for hp in range(2):
                        fillers = list(fillers0 if hp == 0 else fillers1)
                        rate = len(fillers) / max(1, nsk - 2)
                        # start with one filler's credit banked: the extra PE
                        # work at pair start covers the previous pair's
                        # PSUM-release latency before the first PV
                        credit = 1.0
                        po = psO.tile([128, 2 * QW], f32, tag="pvacc",
                                      name=f"po_{hp}_{qb}")
                        prts = {}
                        for kt in range(nsk):
                            c0 = max(0, kt * 128 - qb * QW)
                            cw = QW - c0
                            strip = psS.tile([128, 2 * QW], f32, tag="score",
                                             name=f"sc_{hp}_{qb}_{kt}")
                            nc.tensor.matmul(
                                strip[:, 0:cw],
                                kTt[hp][kt // 4][0:64,
                                                 (kt % 4) * 128:
                                                 (kt % 4) * 128 + 128],
                                qT[hp][qb][0:64, c0:QW],
                                start=True, stop=True,
                                tile_position=(0, 0))
                            nc.tensor.matmul(
                                strip[:, QW:QW + cw],
                                kTt[hp][kt // 4][64:128,
                                                 (kt % 4) * 128:
                                                 (kt % 4) * 128 + 128],
                                qT[hp][qb][64:128, c0:QW],
                                start=True, stop=True,
                                tile_position=(64, 0))
                            prt = wp.tile([128, 2 * QW], MMDT, tag="probs",
                                          name=f"pr_{hp}_{qb}_{kt}")
                            sview = strip[:].rearrange(
                                "p (s q) -> p s q", q=QW)[:, :, 0:cw]
                            pview = prt[:].rearrange(
                                "p (s q) -> p s q", q=QW)[:, :, 0:cw]
                            nc.scalar.activation(pview, sview, Exp,
                                                 scale=0.125)
                            if kt >= nsk - 4:
                                nc.gpsimd.affine_select(
                                    out=pview, in_=pview,
                                    pattern=[[0, 2], [1, cw]], base=0,
                                    channel_multiplier=-1,
                                    compare_op=mybir.AluOpType.is_ge,
                                    fill=0.0)
                            prts[kt] = (prt, c0, cw)
                            if kt < nsk - 2:
                                credit += rate
                                while credit >= 1.0 and fillers:
                                    credit -= 1.0
                                    fillers.pop(0)()
                            if kt > 0:
                                emit_pv(po, hp, qb, kt - 1, nsk, prts)
                        emit_pv(po, hp, qb, nsk - 1, nsk, prts)
                        # pair end: PSUM-freeing copies run first (ACT rows
                        # 0:64 in parallel with the DVE den-row copy), then
                        # the reciprocal + broadcast + normalize multiplies
                        poc = sp.tile([128, 2 * QW], f32, tag="poc")
                        nc.scalar.activation(
                            poc[0:HD, :], po[0:HD, :], Copy)
                        den1 = sp.tile([1, 2 * QW], f32, tag="den1")
                        nc.vector.tensor_copy(den1[:], po[HD:HD + 1, :])
                        rc1 = sp.tile([1, 2 * QW], f32, tag="rc1")
                        nc.vector.reciprocal_approx_fast(
                            out=rc1[:], in_=den1[:])
                        rcb = sp.tile([64, 2 * QW], f32, tag="rcb")
                        nc.gpsimd.partition_broadcast(rcb[:], rc1[:])
                        nc.vector.tensor_tensor(
                            out=attnT[hp][qb][0:64, :],
                            in0=poc[0:HD, 0:QW], in1=rcb[:, 0:QW],
                            op=mybir.AluOpType.mult)
                        nc.vector.tensor_tensor(
                            out=attnT[hp][qb][64:128, :],
                            in0=poc[0:HD, QW:2 * QW], in1=rcb[:, QW:2 * QW],
                            op=mybir.AluOpType.mult)
                        # leftover fillers flush AFTER the normalize ops so
                        # their DVE work cannot delay the PSUM release
                        for f in fillers:
                            f()

                # ---- main wavefront ------------------------------------
                def qk_units(sb, hps=(0, 1)):
                    fs = []
                    for hp in hps:
                        for w_src, dst in ((wq_s, qT), (wk_s, kTt)):
                            fs.append(lambda w=w_src, d=dst, h=hp, s=sb:
                                      emit_qk_proj(w, d, h, s))
                    return fs

                def v_units(sb):
                    return [lambda t=st: emit_v(t)
                            for st in range(sb * 4, sb * 4 + 4)]

                def out_units(sb, on_act=False):
                    return [lambda t=st, d=db: emit_out(t, d, on_act)
                            for st in range(sb * 4, sb * 4 + 4)
                            for db in range(2)]

                def interleave(a, b):
                    fs, a, b = [], list(a), list(b)
                    while a or b:
                        if a:
                            fs.append(a.pop(0))
                        if b:
                            fs.append(b.pop(0))
                    return fs

                # block 0 projections inline
                for f in interleave(qk_units(0), v_units(0)):
                    f()
                # trigger x1/wo loads only now, gated behind qT[0][0]: keeps
                # the first-needed loads alone on the DMA fabric early on
                gate = sp.tile([128, 1], MMDT, tag="gate")
                nc.scalar.activation(gate[:], qT[0][0][:, 0:1], Copy)
                load_x_block(1, eng=nc.scalar)
                nc.scalar.dma_start(wo_s[0][:], woT[0:128, :])
                nc.scalar.dma_start(wo_s[1][:], woT[128:256, :])
                load_x_block(2)
                # attn(0): fillers = projections of block 1
                emit_attention(0,
                               interleave(qk_units(1), v_units(1))[:4],
                               interleave(qk_units(1), v_units(1))[4:])
                load_x_block(3)
                # attn(1): projections of block 2 + out-proj of block 0 (ACT)
                p2 = interleave(qk_units(2), v_units(2))
                o0 = out_units(0)
                emit_attention(1, p2[:4] + o0[:2], p2[4:] + o0[2:])
                # attn(2): hp0 projections of block 3 + v3 + out-proj blk 1
                p3a = interleave(qk_units(3, hps=(0,)), v_units(3))
                o1 = out_units(1)
                emit_attention(2, p3a[:3] + o1[:4], p3a[3:] + o1[4:])
                # attn(3): pair0 gets hp1 projections of blk 3 + out blk 2;
                # pair1 gets the hp0 half of block 3's own out-projection
                o2 = out_units(2)
                hp0f = [lambda t=st, d=db: emit_out_hp0(t, d)
                        for st in range(12, 16) for db in range(2)]
                emit_attention(3, qk_units(3, hps=(1,)) + o2, hp0f)
                for st in range(12, 16):
                    for db in range(2):
                        emit_out_hp1(st, db)

    nc.compile()
    return nc


def _rope_permute_cols(wT):
    """Permute the 64-dh blocks of the [D, GC] transposed weight so RoPE
    pairs (d, d+32) become adjacent columns (2i, 2i+1)."""
    w = wT.reshape(D, GH, HD).copy()
    perm = np.empty(HD, dtype=np.int64)
    perm[0::2] = np.arange(32)
    perm[1::2] = np.arange(32) + 32
    return np.ascontiguousarray(w[:, :, perm].reshape(D, GC))


def _shard_inputs(x, cos, sin, wq, wk, wv, wo):
    perm = np.empty(HD, dtype=np.int64)
    perm[0::2] = np.arange(32)
    perm[1::2] = np.arange(32) + 32
    cosP = np.asarray(cos, np.float32).reshape(S, HD)[:, perm].T  # [64, S]
    sinP = np.asarray(sin, np.float32).reshape(S, HD)[:, perm].T
    snF = sinP.copy()
    # snH = swap(sign-folded sin): the kernel computes swap(pq*snH), so the
    # -sin that lands on even output rows must sit on odd table rows.
    snF[1::2, :] *= -1.0
    cs2 = np.ascontiguousarray(
        np.concatenate([cosP, cosP], axis=0), dtype=np.float16)
    sn2 = np.ascontiguousarray(
        np.concatenate([snF, snF], axis=0), dtype=np.float16)
    x = np.asarray(x, np.float32)

    def to_sbuf_w(wT):
        # [D, GC] -> [128, KT*GC] in the on-chip layout (kt-major per line)
        return np.ascontiguousarray(
            wT.reshape(KT, 128, GC).transpose(1, 0, 2).reshape(128, KT * GC),
            dtype=np.float16)

    def to_sbuf_x(xTb):
        # [D, S] -> [128, QB*KT*QW]: per block cb, kt-major 512-col slices
        v = xTb.reshape(KT, 128, QB, QW).transpose(1, 2, 0, 3)
        return np.ascontiguousarray(
            v.reshape(128, QB * KT * QW), dtype=np.float16)

    in_maps = []
    for c in range(NCORES):
        b, g = c // GROUPS, c % GROUPS
        rows = slice(g * GC, (g + 1) * GC)
        wqTg = _rope_permute_cols(np.asarray(wq, np.float32)[rows, :].T)
        wkTg = _rope_permute_cols(np.asarray(wk, np.float32)[rows, :].T)
        in_maps.append({
            "xS": to_sbuf_x(x[b].T),
            "wqS": to_sbuf_w(wqTg),
            "wkS": to_sbuf_w(wkTg),
            "wvS": to_sbuf_w(np.asarray(wv, np.float32)[rows, :].T),
            "woT": np.ascontiguousarray(
                np.asarray(wo, np.float32)[:, rows].T, dtype=np.float16),
            "cs2": cs2,
            "sn2": sn2,
        })
    return in_maps


def _run(inputs, trace=False, trace_kwargs=None):
    if "nc" not in _cache:
        _cache["nc"] = _build()
    nc = _cache["nc"]
    in_maps = _shard_inputs(
        inputs["x"], inputs["cos"], inputs["sin"],
        inputs["wq"], inputs["wk"], inputs["wv"], inputs["wo"])
    res = run_bass_kernel_spmd(
        nc, in_maps, list(range(NCORES)), trace=trace,
        **(trace_kwargs or {}))
    full = np.zeros((B, S, D), dtype=np.float32)
    for c in range(NCORES):
        full[c // GROUPS] += res.results[c]["out"].astype(np.float32)
    return full, res


def kernel(**inputs):
    full, _ = _run(inputs, trace=False)
    return full


# revision 23
# speedup vs baseline: 1.1522x; 1.1522x over previous
"""Trainium2 Bass kernel for nn_Attention_84473416778449.

Reference computation (B=2, S=2048, D=1024, H=16, HD=64, fp32):
    q/k/v = x @ w{q,k,v}.T ; RoPE(q, k) ; causal softmax attention ; out @ wo.T

Sharding: 8 cores = (batch 2) x (head-group 4). Each core computes 4 heads of
one batch end-to-end and a partial output projection over its 256 channels;
the host sums the 4 partials per batch.

Key design points (v3):
  - All device inputs fp16 (host pre-converts); output fp16, summed on host.
  - RoPE without a PE matmul: host permutes q/k output channels so RoPE
    pairs (d, d+32) sit adjacent; rotate_half becomes a DVE stream_shuffle
    (adjacent-pair swap within 32-blocks), signs folded into the sin table.
  - Scores for the two heads of a pair run concurrently on the PE via row
    tiling (tile_position (0,0) / (64,0)), writing halves of one
    [128, 1024] PSUM strip consumed by a single batched exp per kt.
  - Wavefront + fillers: projections of later blocks and output projections
    of earlier blocks are interleaved between attention matmul groups so no
    engine idles long; filler lists are balanced per phase.
  - Pair-boundary decoupling: at each pair end only the two PSUM-freeing
    copies run immediately (ACT: PV rows, DVE: denominator row); the
    reciprocal/broadcast/normalize-multiplies are deferred into the next
    pair's iterations.
  - Final block output projection is hp-split: the attnT[0] half runs as
    fillers during the last pair; only the attnT[1] matmul + add + DMA
    remain in the tail.
  - PSUM (8 banks): proj 2 + score strips 4 + PV accumulator pair 2.
"""
import sys

if "/opt/trn_rl_repo" not in sys.path:
    sys.path.insert(0, "/opt/trn_rl_repo")

import numpy as np

import concourse.bass as bass
import concourse.mybir as mybir
import concourse.tile as tile
from concourse import bacc
from concourse.bass_utils import run_bass_kernel_spmd

B, S, D, H, HD = 2, 2048, 1024, 16, 64
NCORES = 8
GROUPS = 4            # head groups
GH = H // GROUPS      # heads per group = 4
GC = GH * HD          # channels per group = 256
KT = D // 128         # 8 k-tiles over D
ST = S // 128         # 16 s-tiles
QB = 4                # sq blocks of 512
QW = S // QB          # 512
VW = GH * (HD + 1)    # 260: v tile payload columns
VPAD = 3 * (HD + 1) + 128   # 323: pad so stationary window fits for h=3

f32 = mybir.dt.float32
MMDT = mybir.dt.float16   # matmul-operand dtype
Exp = mybir.ActivationFunctionType.Exp
Copy = mybir.ActivationFunctionType.Copy

# stream_shuffle mask: swap adjacent pairs within each 32-partition quadrant
SWAP_MASK = [j ^ 1 for j in range(32)]

# stride-0 partition APs are rejected by the DVE ("partition dimension must
# have nonzero step") — go through the gpsimd partition_broadcast
USE_BCAST_AP = False

_cache = {}


def _build():
    nc = bacc.Bacc("TRN2", num_devices=NCORES)

    # x and q/k/v weights arrive pre-arranged in the exact SBUF layout so
    # every load is a contiguous full-rate DMA (8KB / 4KB partition lines)
    xS = nc.dram_tensor("xS", [128, QB * KT * QW], MMDT,
                        kind="ExternalInput").ap()
    wqS = nc.dram_tensor("wqS", [128, KT * GC], MMDT,
                         kind="ExternalInput").ap()
    wkS = nc.dram_tensor("wkS", [128, KT * GC], MMDT,
                         kind="ExternalInput").ap()
    wvS = nc.dram_tensor("wvS", [128, KT * GC], MMDT,
                         kind="ExternalInput").ap()
    woT = nc.dram_tensor("woT", [GC, D], MMDT, kind="ExternalInput").ap()
    cs2 = nc.dram_tensor("cs2", [128, S], MMDT, kind="ExternalInput").ap()
    sn2 = nc.dram_tensor("sn2", [128, S], MMDT, kind="ExternalInput").ap()
    out = nc.dram_tensor("out", [S, D], MMDT, kind="ExternalOutput").ap()

    with tile.TileContext(nc) as tc:
        with tc.tile_pool(name="persist", bufs=1) as pp, \
             tc.tile_pool(name="rope", bufs=3) as rp, \
             tc.tile_pool(name="probs", bufs=4) as wp, \
             tc.tile_pool(name="outsb", bufs=3) as op_, \
             tc.tile_pool(name="ob0s", bufs=8) as o0p, \
             tc.tile_pool(name="norm", bufs=2) as sp:

            # ---- first-needed loads go out before anything else ------------
            xTb = [pp.tile([128, KT * QW], MMDT, tag=f"xTb{cb}",
                           name=f"xTb{cb}") for cb in range(QB)]

            def xTr(kt, cb):
                return xTb[cb][:, kt * QW:(kt + 1) * QW]

            def load_x_block(cb, chunks=1, eng=None):
                eng = eng or nc.gpsimd
                w = KT * QW // chunks
                for j in range(chunks):
                    eng.dma_start(
                        xTb[cb][:, j * w:(j + 1) * w],
                        xS[:, cb * KT * QW + j * w:cb * KT * QW + (j + 1) * w])

            def load_wT(srcS, eng):
                t = pp.tile([128, KT * GC], MMDT, tag=f"w{srcS.tensor.name}",
                            name=f"w{srcS.tensor.name}")
                eng.dma_start(t[:], srcS[:])
                return t

            # spread the startup loads over the three DMA-capable queues;
            # x1 and wo are triggered later, gated behind compute sems, so
            # the first-needed loads monopolize early DMA bandwidth
            wq_s = load_wT(wqS, nc.sync)
            load_x_block(0, chunks=4)
            wk_s = load_wT(wkS, nc.scalar)
            cs_sb = pp.tile([128, S], MMDT, tag="cs")
            nc.sync.dma_start(cs_sb[:], cs2[:])
            sn_sb = pp.tile([128, S], MMDT, tag="sn")
            nc.sync.dma_start(sn_sb[:], sn2[:])
            wv_s = load_wT(wvS, nc.scalar)
            wo_s = []
            for kt in range(2):
                t = pp.tile([128, D], MMDT, tag=f"wo{kt}", name=f"wo{kt}")
                wo_s.append(t)

            qT = [[pp.tile([128, QW], MMDT, tag=f"qT{i}_{b}",
                           name=f"qT{i}_{b}") for b in range(QB)]
                  for i in range(2)]
            kTt = [[pp.tile([128, QW], MMDT, tag=f"kT{i}_{b}",
                            name=f"kT{i}_{b}") for b in range(QB)]
                   for i in range(2)]
            attnT = [[pp.tile([128, QW], MMDT, tag=f"aT{i}_{b}",
                              name=f"aT{i}_{b}") for b in range(QB)]
                     for i in range(2)]
            v_sb = [pp.tile([128, VPAD], MMDT, tag=f"v{i}",
                            name=f"v{i}") for i in range(ST)]
            # one-time v-tile framing: ones column per head (the PV
            # denominator lands in psum row 64; engine partition bases must
            # be 32-aligned so row 64 is staged via a 1-partition DVE copy)
            for st in range(ST):
                vhe = v_sb[st][:, 0:VW].rearrange("p (h e) -> p h e", e=HD + 1)
                nc.vector.memset(vhe[:, :, HD:HD + 1], 1.0)
                nc.vector.memset(v_sb[st][:, VW:VPAD], 0.0)

            with tc.tile_pool(name="psP", bufs=2, space="PSUM") as psP, \
                 tc.tile_pool(name="psS", bufs=2, space="PSUM") as psS, \
                 tc.tile_pool(name="psO", bufs=1, space="PSUM") as psO:

                def emit_qk_proj(w_src, dst, hp, sb):
                    cols = slice(sb * QW, (sb + 1) * QW)
                    pq = psP.tile([128, QW], f32, tag="proj",
                                  name=f"pq_{0 if dst is qT else 1}_{hp}_{sb}")
                    for kt in range(KT):
                        nc.tensor.matmul(
                            pq[:],
                            w_src[:, kt * GC + hp * 128:
                                  kt * GC + hp * 128 + 128],
                            xTr(kt, sb),
                            start=(kt == 0), stop=(kt == KT - 1))
                    # RoPE: dst = pqh*cos + swap(pqh*snH); snH is the
                    # pre-swapped sign-folded sin table (host-built); swap is
                    # the adjacent-pair partition shuffle (fp16 in/out).
                    # pq is staged to fp16 SBUF first (DVE copy, 2x mode):
                    # this frees the PSUM bank quickly and lets the DVE
                    # multiplies run in 2x packed mode.
                    pqh = rp.tile([128, QW], MMDT, tag="pqh")
                    nc.vector.tensor_copy(pqh[:], pq[:])
                    tcs = rp.tile([128, QW], MMDT, tag="tcs")
                    nc.vector.tensor_tensor(
                        out=tcs[:], in0=pqh[:], in1=cs_sb[:, cols],
                        op=mybir.AluOpType.mult)
                    tsp = rp.tile([128, QW], MMDT, tag="tsp")
                    nc.vector.tensor_tensor(
                        out=tsp[:], in0=pqh[:], in1=sn_sb[:, cols],
                        op=mybir.AluOpType.mult)
                    tsn = rp.tile([128, QW], MMDT, tag="tsn")
                    nc.vector.stream_shuffle(tsn[:], tsp[:], SWAP_MASK)
                    nc.vector.tensor_tensor(
                        out=dst[hp][sb][:], in0=tcs[:], in1=tsn[:],
                        op=mybir.AluOpType.add)

                def emit_v(st):
                    pv = psP.tile([128, QW], f32, tag="proj",
                                  name=f"pv_{st}")
                    for kt in range(KT):
                        nc.tensor.matmul(
                            pv[:, 0:GC],
                            xTr(kt, st // 4)[:, (st % 4) * 128:
                                             (st % 4) * 128 + 128],
                            wv_s[:, kt * GC:(kt + 1) * GC],
                            start=(kt == 0), stop=(kt == KT - 1))
                    vhe = v_sb[st][:, 0:VW].rearrange(
                        "p (h e) -> p h e", e=HD + 1)
                    nc.vector.tensor_copy(
                        vhe[:, :, 0:HD],
                        pv[:, 0:GC].rearrange("p (h d) -> p h d", d=HD))

                def emit_out(st, db, on_act=False):
                    pc = psP.tile([128, QW], f32, tag="proj",
                                  name=f"pc_{st}_{db}")
                    for hp in range(2):
                        nc.tensor.matmul(
                            pc[:],
                            attnT[hp][st // 4][:, (st % 4) * 128:
                                               (st % 4) * 128 + 128],
                            wo_s[hp][:, db * QW:(db + 1) * QW],
                            start=(hp == 0), stop=(hp == 1))
                    ob = op_.tile([128, QW], MMDT, tag="outsb")
                    if on_act:
                        nc.scalar.activation(ob[:], pc[:], Copy)
                    else:
                        nc.vector.tensor_copy(ob[:], pc[:])
                    nc.sync.dma_start(
                        out[st * 128:(st + 1) * 128,
                            db * QW:(db + 1) * QW], ob[:])

                # hp-split output projection for the final block
                ob0s = {}

                def emit_out_hp0(st, db):
                    pc = psP.tile([128, QW], f32, tag="proj",
                                  name=f"pc0_{st}_{db}")
                    nc.tensor.matmul(
                        pc[:],
                        attnT[0][st // 4][:, (st % 4) * 128:
                                          (st % 4) * 128 + 128],
                        wo_s[0][:, db * QW:(db + 1) * QW],
                        start=True, stop=True)
                    ob0 = o0p.tile([128, QW], MMDT, tag="ob0",
                                   name=f"ob0_{st}_{db}")
                    nc.vector.tensor_copy(ob0[:], pc[:])
                    ob0s[(st, db)] = ob0

                def emit_out_hp1(st, db):
                    pc = psP.tile([128, QW], f32, tag="proj",
                                  name=f"pc1_{st}_{db}")
                    nc.tensor.matmul(
                        pc[:],
                        attnT[1][st // 4][:, (st % 4) * 128:
                                          (st % 4) * 128 + 128],
                        wo_s[1][:, db * QW:(db + 1) * QW],
                        start=True, stop=True)
                    ob = op_.tile([128, QW], MMDT, tag="outsb")
                    nc.vector.tensor_tensor(
                        out=ob[:], in0=pc[:], in1=ob0s[(st, db)][:],
                        op=mybir.AluOpType.add)
                    nc.sync.dma_start(
                        out[st * 128:(st + 1) * 128,
                            db * QW:(db + 1) * QW], ob[:])


                def emit_pv(po, hp, qb, kt, nsk, prts):
                    prt, c0, cw = prts.pop(kt)
                    for i, h in enumerate((2 * hp, 2 * hp + 1)):
                        nc.tensor.matmul(
                            po[:, i * QW + c0:(i + 1) * QW],
                            v_sb[kt][:, h * (HD + 1):h * (HD + 1) + 128],
                            prt[:, i * QW:i * QW + cw],
                            start=(kt == 0), stop=(kt == nsk - 1))

                def emit_attention(qb, fillers0=(), fillers1=()):
                    nsk = (qb + 1) * 4
                    for hp in range(2):
                        fillers = list(fillers0 if hp == 0 else fillers1)
                        rate = len(fillers) / max(1, nsk - 2)
                        # start with one filler's credit banked: the extra PE
                        # work at pair start covers the previous pair's
                        # PSUM-release latency before the first PV
                        credit = 1.0
                        po = psO.tile([128, 2 * QW], f32, tag="pvacc",
                                      name=f"po_{hp}_{qb}")
                        prts = {}
                        for kt in range(nsk):
                            c0 = max(0, kt * 128 - qb * QW)
                            cw = QW - c0
                            strip = psS.tile([128, 2 * QW], f32, tag="score",
                                             name=f"sc_{hp}_{qb}_{kt}")
                            nc.tensor.matmul(
                                strip[:, 0:cw],
                                kTt[hp][kt // 4][0:64,
                                                 (kt % 4) * 128:
                                                 (kt % 4) * 128 + 128],
                                qT[hp][qb][0:64, c0:QW],
                                start=True, stop=True,
                                tile_position=(0, 0))
                            nc.tensor.matmul(
                                strip[:, QW:QW + cw],
                                kTt[hp][kt // 4][64:128,
                                                 (kt % 4) * 128:
                                                 (kt % 4) * 128 + 128],
                                qT[hp][qb][64:128, c0:QW],
                                start=True, stop=True,
                                tile_position=(64, 0))
                            prt = wp.tile([128, 2 * QW], MMDT, tag="probs",
                                          name=f"pr_{hp}_{qb}_{kt}")
                            sview = strip[:].rearrange(
                                "p (s q) -> p s q", q=QW)[:, :, 0:cw]
                            pview = prt[:].rearrange(
                                "p (s q) -> p s q", q=QW)[:, :, 0:cw]
                            nc.scalar.activation(pview, sview, Exp,
                                                 scale=0.125)
                            if kt >= nsk - 4:
                                nc.gpsimd.affine_select(
                                    out=pview, in_=pview,
                                    pattern=[[0, 2], [1, cw]], base=0,
                                    channel_multiplier=-1,
                                    compare_op=mybir.AluOpType.is_ge,
                                    fill=0.0)
                            prts[kt] = (prt, c0, cw)
                            if kt < nsk - 2:
                                credit += rate
                                while credit >= 1.0 and fillers:
                                    credit -= 1.0
                                    fillers.pop(0)()
                            if kt > 0:
                                emit_pv(po, hp, qb, kt - 1, nsk, prts)
                        emit_pv(po, hp, qb, nsk - 1, nsk, prts)
                        # pair end: PSUM-freeing copies run first (ACT rows
                        # 0:64 in parallel with the DVE den-row copy), then
                        # the reciprocal + broadcast + normalize multiplies
                        poc = sp.tile([128, 2 * QW], f32, tag="poc")
                        nc.scalar.activation(
                            poc[0:HD, :], po[0:HD, :], Copy)
                        den1 = sp.tile([1, 2 * QW], f32, tag="den1")
                        nc.vector.tensor_copy(den1[:], po[HD:HD + 1, :])
                        rc1 = sp.tile([1, 2 * QW], f32, tag="rc1")
                        nc.vector.reciprocal_approx_fast(
                            out=rc1[:], in_=den1[:])
                        rcb = sp.tile([64, 2 * QW], f32, tag="rcb")
                        nc.gpsimd.partition_broadcast(rcb[:], rc1[:])
                        nc.vector.tensor_tensor(
                            out=attnT[hp][qb][0:64, :],
                            in0=poc[0:HD, 0:QW], in1=rcb[:, 0:QW],
                            op=mybir.AluOpType.mult)
                        nc.vector.tensor_tensor(
                            out=attnT[hp][qb][64:128, :],
                            in0=poc[0:HD, QW:2 * QW], in1=rcb[:, QW:2 * QW],
                            op=mybir.AluOpType.mult)
                        # leftover fillers flush AFTER the normalize ops so
                        # their DVE work cannot delay the PSUM release
                        for f in fillers:
                            f()

                # ---- main wavefront ------------------------------------
                def qk_units(sb, hps=(0, 1)):
                    fs = []
                    for hp in hps:
                        for w_src, dst in ((wq_s, qT), (wk_s, kTt)):
                            fs.append(lambda w=w_src, d=dst, h=hp, s=sb:
                                      emit_qk_proj(w, d, h, s))
                    return fs

                def v_units(sb):
                    return [lambda t=st: emit_v(t)
                            for st in range(sb * 4, sb * 4 + 4)]

                def out_units(sb, on_act=False):
                    return [lambda t=st, d=db: emit_out(t, d, on_act)
                            for st in range(sb * 4, sb * 4 + 4)
                            for db in range(2)]

                def interleave(a, b):
                    fs, a, b = [], list(a), list(b)
                    while a or b:
                        if a:
                            fs.append(a.pop(0))
                        if b:
                            fs.append(b.pop(0))
                    return fs

                # block 0 projections inline
                for f in interleave(qk_units(0), v_units(0)):
                    f()
                # trigger x1/wo loads only now, gated behind qT[0][0]: keeps
                # the first-needed loads alone on the DMA fabric early on
                gate = sp.tile([128, 1], MMDT, tag="gate")
                nc.scalar.activation(gate[:], qT[0][0][:, 0:1], Copy)
                load_x_block(1, eng=nc.scalar)
                nc.scalar.dma_start(wo_s[0][:], woT[0:128, :])
                nc.scalar.dma_start(wo_s[1][:], woT[128:256, :])
                load_x_block(2)
                # attn(0): fillers = projections of block 1
                emit_attention(0,
                               interleave(qk_units(1), v_units(1))[:4],
                               interleave(qk_units(1), v_units(1))[4:])
                load_x_block(3)
                # attn(1): projections of block 2 + out-proj of block 0 (ACT)
                p2 = interleave(qk_units(2), v_units(2))
                o0 = out_units(0)
                emit_attention(1, p2[:4] + o0[:2], p2[4:] + o0[2:])
                # attn(2): hp0 projections of block 3 + v3 + out-proj blk 1
                p3a = interleave(qk_units(3, hps=(0,)), v_units(3))
                o1 = out_units(1)
                emit_attention(2, p3a[:3] + o1[:4], p3a[3:] + o1[4:])
                # attn(3): pair0 gets hp1 projections of blk 3 + out blk 2;
                # pair1 gets the hp0 half of block 3's own out-projection
                o2 = out_units(2)
                hp0f = [lambda t=st, d=db: emit_out_hp0(t, d)
                        for st in range(12, 16) for db in range(2)]
                emit_attention(3, qk_units(3, hps=(1,)) + o2, hp0f)
                for st in range(12, 16):
                    for db in range(2):
                        emit_out_hp1(st, db)

    nc.compile()
    return nc


def _rope_permute_cols(wT):
    """Permute the 64-dh blocks of the [D, GC] transposed weight so RoPE
    pairs (d, d+32) become adjacent columns (2i, 2i+1)."""
    w = wT.reshape(D, GH, HD).copy()
    perm = np.empty(HD, dtype=np.int64)
    perm[0::2] = np.arange(32)
    perm[1::2] = np.arange(32) + 32
    return np.ascontiguousarray(w[:, :, perm].reshape(D, GC))


def _shard_inputs(x, cos, sin, wq, wk, wv, wo):
    perm = np.empty(HD, dtype=np.int64)
    perm[0::2] = np.arange(32)
    perm[1::2] = np.arange(32) + 32
    cosP = np.asarray(cos, np.float32).reshape(S, HD)[:, perm].T  # [64, S]
    sinP = np.asarray(sin, np.float32).reshape(S, HD)[:, perm].T
    snF = sinP.copy()
    # snH = swap(sign-folded sin): the kernel computes swap(pq*snH), so the
    # -sin that lands on even output rows must sit on odd table rows.
    snF[1::2, :] *= -1.0
    cs2 = np.ascontiguousarray(
        np.concatenate([cosP, cosP], axis=0), dtype=np.float16)
    sn2 = np.ascontiguousarray(
        np.concatenate([snF, snF], axis=0), dtype=np.float16)
    x = np.asarray(x, np.float32)

    def to_sbuf_w(wT):
        # [D, GC] -> [128, KT*GC] in the on-chip layout (kt-major per line)
        return np.ascontiguousarray(
            wT.reshape(KT, 128, GC).transpose(1, 0, 2).reshape(128, KT * GC),
            dtype=np.float16)

    def to_sbuf_x(xTb):
        # [D, S] -> [128, QB*KT*QW]: per block cb, kt-major 512-col slices
        v = xTb.reshape(KT, 128, QB, QW).transpose(1, 2, 0, 3)
        return np.ascontiguousarray(
            v.reshape(128, QB * KT * QW), dtype=np.float16)

    in_maps = []
    for c in range(NCORES):
        b, g = c // GROUPS, c % GROUPS
        rows = slice(g * GC, (g + 1) * GC)
        wqTg = _rope_permute_cols(np.asarray(wq, np.float32)[rows, :].T)
        wkTg = _rope_permute_cols(np.asarray(wk, np.float32)[rows, :].T)
        in_maps.append({
            "xS": to_sbuf_x(x[b].T),
            "wqS": to_sbuf_w(wqTg),
            "wkS": to_sbuf_w(wkTg),
            "wvS": to_sbuf_w(np.asarray(wv, np.float32)[rows, :].T),
            "woT": np.ascontiguousarray(
                np.asarray(wo, np.float32)[:, rows].T, dtype=np.float16),
            "cs2": cs2,
            "sn2": sn2,
        })
    return in_maps


def _run(inputs, trace=False, trace_kwargs=None):
    if "nc" not in _cache:
        _cache["nc"] = _build()
    nc = _cache["nc"]
    in_maps = _shard_inputs(
        inputs["x"], inputs["cos"], inputs["sin"],
        inputs["wq"], inputs["wk"], inputs["wv"], inputs["wo"])
    res = run_bass_kernel_spmd(
        nc, in_maps, list(range(NCORES)), trace=trace,
        **(trace_kwargs or {}))
    full = np.zeros((B, S, D), dtype=np.float32)
    for c in range(NCORES):
        full[c // GROUPS] += res.results[c]["out"].astype(np.float32)
    return full, res


def kernel(**inputs):
    full, _ = _run(inputs, trace=False)
    return full


# revision 27
# speedup vs baseline: 1.1568x; 1.0040x over previous
"""Trainium2 Bass kernel for nn_Attention_84473416778449.

Reference computation (B=2, S=2048, D=1024, H=16, HD=64, fp32):
    q/k/v = x @ w{q,k,v}.T ; RoPE(q, k) ; causal softmax attention ; out @ wo.T

Sharding: 8 cores = (batch 2) x (head-group 4). Each core computes 4 heads of
one batch end-to-end and a partial output projection over its 256 channels;
the host sums the 4 partials per batch.

Key design points (v3):
  - All device inputs fp16 (host pre-converts); output fp16, summed on host.
  - RoPE without a PE matmul: host permutes q/k output channels so RoPE
    pairs (d, d+32) sit adjacent; rotate_half becomes a DVE stream_shuffle
    (adjacent-pair swap within 32-blocks), signs folded into the sin table.
  - Scores for the two heads of a pair run concurrently on the PE via row
    tiling (tile_position (0,0) / (64,0)), writing halves of one
    [128, 1024] PSUM strip consumed by a single batched exp per kt.
  - Wavefront + fillers: projections of later blocks and output projections
    of earlier blocks are interleaved between attention matmul groups so no
    engine idles long; filler lists are balanced per phase.
  - Pair-boundary decoupling: at each pair end only the two PSUM-freeing
    copies run immediately (ACT: PV rows, DVE: denominator row); the
    reciprocal/broadcast/normalize-multiplies are deferred into the next
    pair's iterations.
  - Final block output projection is hp-split: the attnT[0] half runs as
    fillers during the last pair; only the attnT[1] matmul + add + DMA
    remain in the tail.
  - PSUM (8 banks): proj 2 + score strips 4 + PV accumulator pair 2.
"""
import sys

if "/opt/trn_rl_repo" not in sys.path:
    sys.path.insert(0, "/opt/trn_rl_repo")

import numpy as np

import concourse.bass as bass
import concourse.mybir as mybir
import concourse.tile as tile
from concourse import bacc
from concourse.bass_utils import run_bass_kernel_spmd

B, S, D, H, HD = 2, 2048, 1024, 16, 64
NCORES = 8
GROUPS = 4            # head groups
GH = H // GROUPS      # heads per group = 4
GC = GH * HD          # channels per group = 256
KT = D // 128         # 8 k-tiles over D
ST = S // 128         # 16 s-tiles
QB = 4                # sq blocks of 512
QW = S // QB          # 512
VW = GH * (HD + 1)    # 260: v tile payload columns
VPAD = 3 * (HD + 1) + 128   # 323: pad so stationary window fits for h=3

f32 = mybir.dt.float32
MMDT = mybir.dt.float16   # matmul-operand dtype
Exp = mybir.ActivationFunctionType.Exp
Copy = mybir.ActivationFunctionType.Copy

# stream_shuffle mask: swap adjacent pairs within each 32-partition quadrant
SWAP_MASK = [j ^ 1 for j in range(32)]

# stride-0 partition APs are rejected by the DVE ("partition dimension must
# have nonzero step") — go through the gpsimd partition_broadcast
USE_BCAST_AP = False

_cache = {}


def _build():
    nc = bacc.Bacc("TRN2", num_devices=NCORES)

    # x and q/k/v weights arrive pre-arranged in the exact SBUF layout so
    # every load is a contiguous full-rate DMA (8KB / 4KB partition lines)
    xS = nc.dram_tensor("xS", [128, QB * KT * QW], MMDT,
                        kind="ExternalInput").ap()
    wqS = nc.dram_tensor("wqS", [128, KT * GC], MMDT,
                         kind="ExternalInput").ap()
    wkS = nc.dram_tensor("wkS", [128, KT * GC], MMDT,
                         kind="ExternalInput").ap()
    wvS = nc.dram_tensor("wvS", [128, KT * GC], MMDT,
                         kind="ExternalInput").ap()
    woT = nc.dram_tensor("woT", [GC, D], MMDT, kind="ExternalInput").ap()
    cs2 = nc.dram_tensor("cs2", [128, S], MMDT, kind="ExternalInput").ap()
    sn2 = nc.dram_tensor("sn2", [128, S], MMDT, kind="ExternalInput").ap()
    out = nc.dram_tensor("out", [S, D], MMDT, kind="ExternalOutput").ap()
    # block-3 rows get an hp-split output: the wo[0:128] half lands in `out`,
    # the wo[128:256] half in `out3b`; the host sums them
    out3b = nc.dram_tensor("out3b", [QW, D], MMDT,
                           kind="ExternalOutput").ap()

    with tile.TileContext(nc) as tc:
        with tc.tile_pool(name="persist", bufs=1) as pp, \
             tc.tile_pool(name="rope", bufs=3) as rp, \
             tc.tile_pool(name="probs", bufs=4) as wp, \
             tc.tile_pool(name="outsb", bufs=3) as op_, \
             tc.tile_pool(name="norm", bufs=2) as sp:

            # ---- first-needed loads go out before anything else ------------
            xTb = [pp.tile([128, KT * QW], MMDT, tag=f"xTb{cb}",
                           name=f"xTb{cb}") for cb in range(QB)]

            def xTr(kt, cb):
                return xTb[cb][:, kt * QW:(kt + 1) * QW]

            def load_x_block(cb, chunks=1, eng=None):
                eng = eng or nc.gpsimd
                w = KT * QW // chunks
                for j in range(chunks):
                    eng.dma_start(
                        xTb[cb][:, j * w:(j + 1) * w],
                        xS[:, cb * KT * QW + j * w:cb * KT * QW + (j + 1) * w])

            def load_wT(srcS, eng):
                t = pp.tile([128, KT * GC], MMDT, tag=f"w{srcS.tensor.name}",
                            name=f"w{srcS.tensor.name}")
                eng.dma_start(t[:], srcS[:])
                return t

            # spread the startup loads over the three DMA-capable queues;
            # x1 and wo are triggered later, gated behind compute sems, so
            # the first-needed loads monopolize early DMA bandwidth
            wq_s = load_wT(wqS, nc.sync)
            load_x_block(0, chunks=4)
            wk_s = load_wT(wkS, nc.scalar)
            cs_sb = pp.tile([128, S], MMDT, tag="cs")
            nc.sync.dma_start(cs_sb[:], cs2[:])
            sn_sb = pp.tile([128, S], MMDT, tag="sn")
            nc.sync.dma_start(sn_sb[:], sn2[:])
            wv_s = load_wT(wvS, nc.scalar)
            wo_s = []
            for kt in range(2):
                t = pp.tile([128, D], MMDT, tag=f"wo{kt}", name=f"wo{kt}")
                wo_s.append(t)

            qT = [[pp.tile([128, QW], MMDT, tag=f"qT{i}_{b}",
                           name=f"qT{i}_{b}") for b in range(QB)]
                  for i in range(2)]
            kTt = [[pp.tile([128, QW], MMDT, tag=f"kT{i}_{b}",
                            name=f"kT{i}_{b}") for b in range(QB)]
                   for i in range(2)]
            attnT = [[pp.tile([128, QW], MMDT, tag=f"aT{i}_{b}",
                              name=f"aT{i}_{b}") for b in range(QB)]
                     for i in range(2)]
            v_sb = [pp.tile([128, VPAD], MMDT, tag=f"v{i}",
                            name=f"v{i}") for i in range(ST)]
            # one-time v-tile framing: ones column per head (the PV
            # denominator lands in psum row 64; engine partition bases must
            # be 32-aligned so row 64 is staged via a 1-partition DVE copy)
            for st in range(ST):
                vhe = v_sb[st][:, 0:VW].rearrange("p (h e) -> p h e", e=HD + 1)
                nc.vector.memset(vhe[:, :, HD:HD + 1], 1.0)
                nc.vector.memset(v_sb[st][:, VW:VPAD], 0.0)

            with tc.tile_pool(name="psP", bufs=2, space="PSUM") as psP, \
                 tc.tile_pool(name="psS", bufs=2, space="PSUM") as psS, \
                 tc.tile_pool(name="psO", bufs=1, space="PSUM") as psO:

                def emit_qk_proj(w_src, dst, hp, sb):
                    cols = slice(sb * QW, (sb + 1) * QW)
                    pq = psP.tile([128, QW], f32, tag="proj",
                                  name=f"pq_{0 if dst is qT else 1}_{hp}_{sb}")
                    for kt in range(KT):
                        nc.tensor.matmul(
                            pq[:],
                            w_src[:, kt * GC + hp * 128:
                                  kt * GC + hp * 128 + 128],
                            xTr(kt, sb),
                            start=(kt == 0), stop=(kt == KT - 1))
                    # RoPE: dst = pqh*cos + swap(pqh*snH); snH is the
                    # pre-swapped sign-folded sin table (host-built); swap is
                    # the adjacent-pair partition shuffle (fp16 in/out).
                    # pq is staged to fp16 SBUF first (DVE copy, 2x mode):
                    # this frees the PSUM bank quickly and lets the DVE
                    # multiplies run in 2x packed mode.
                    pqh = rp.tile([128, QW], MMDT, tag="pqh")
                    nc.vector.tensor_copy(pqh[:], pq[:])
                    tcs = rp.tile([128, QW], MMDT, tag="tcs")
                    nc.vector.tensor_tensor(
                        out=tcs[:], in0=pqh[:], in1=cs_sb[:, cols],
                        op=mybir.AluOpType.mult)
                    tsp = rp.tile([128, QW], MMDT, tag="tsp")
                    nc.vector.tensor_tensor(
                        out=tsp[:], in0=pqh[:], in1=sn_sb[:, cols],
                        op=mybir.AluOpType.mult)
                    tsn = rp.tile([128, QW], MMDT, tag="tsn")
                    nc.vector.stream_shuffle(tsn[:], tsp[:], SWAP_MASK)
                    nc.vector.tensor_tensor(
                        out=dst[hp][sb][:], in0=tcs[:], in1=tsn[:],
                        op=mybir.AluOpType.add)

                def emit_v(st):
                    pv = psP.tile([128, QW], f32, tag="proj",
                                  name=f"pv_{st}")
                    for kt in range(KT):
                        nc.tensor.matmul(
                            pv[:, 0:GC],
                            xTr(kt, st // 4)[:, (st % 4) * 128:
                                             (st % 4) * 128 + 128],
                            wv_s[:, kt * GC:(kt + 1) * GC],
                            start=(kt == 0), stop=(kt == KT - 1))
                    vhe = v_sb[st][:, 0:VW].rearrange(
                        "p (h e) -> p h e", e=HD + 1)
                    nc.vector.tensor_copy(
                        vhe[:, :, 0:HD],
                        pv[:, 0:GC].rearrange("p (h d) -> p h d", d=HD))

                def emit_out(st, db, on_act=False):
                    pc = psP.tile([128, QW], f32, tag="proj",
                                  name=f"pc_{st}_{db}")
                    for hp in range(2):
                        nc.tensor.matmul(
                            pc[:],
                            attnT[hp][st // 4][:, (st % 4) * 128:
                                               (st % 4) * 128 + 128],
                            wo_s[hp][:, db * QW:(db + 1) * QW],
                            start=(hp == 0), stop=(hp == 1))
                    ob = op_.tile([128, QW], MMDT, tag="outsb")
                    if on_act:
                        nc.scalar.activation(ob[:], pc[:], Copy)
                    else:
                        nc.vector.tensor_copy(ob[:], pc[:])
                    nc.sync.dma_start(
                        out[st * 128:(st + 1) * 128,
                            db * QW:(db + 1) * QW], ob[:])

                # hp-split output projection for the final block: each half
                # is DMA'd as its own partial (host sums), so the tail has no
                # serial DVE adds; the hp1 copies ride the idle ACT engine
                def emit_out_hp0(st, db):
                    pc = psP.tile([128, QW], f32, tag="proj",
                                  name=f"pc0_{st}_{db}")
                    nc.tensor.matmul(
                        pc[:],
                        attnT[0][st // 4][:, (st % 4) * 128:
                                          (st % 4) * 128 + 128],
                        wo_s[0][:, db * QW:(db + 1) * QW],
                        start=True, stop=True)
                    ob = op_.tile([128, QW], MMDT, tag="outsb")
                    nc.vector.tensor_copy(ob[:], pc[:])
                    nc.sync.dma_start(
                        out[st * 128:(st + 1) * 128,
                            db * QW:(db + 1) * QW], ob[:])

                def emit_out_hp1(st, db):
                    pc = psP.tile([128, QW], f32, tag="proj",
                                  name=f"pc1_{st}_{db}")
                    nc.tensor.matmul(
                        pc[:],
                        attnT[1][st // 4][:, (st % 4) * 128:
                                          (st % 4) * 128 + 128],
                        wo_s[1][:, db * QW:(db + 1) * QW],
                        start=True, stop=True)
                    ob = op_.tile([128, QW], MMDT, tag="outsb")
                    nc.scalar.activation(ob[:], pc[:], Copy)
                    nc.sync.dma_start(
                        out3b[(st - 12) * 128:(st - 11) * 128,
                              db * QW:(db + 1) * QW], ob[:])


                def emit_pv(po, hp, qb, kt, nsk, prts):
                    prt, c0, cw = prts.pop(kt)
                    for i, h in enumerate((2 * hp, 2 * hp + 1)):
                        nc.tensor.matmul(
                            po[:, i * QW + c0:(i + 1) * QW],
                            v_sb[kt][:, h * (HD + 1):h * (HD + 1) + 128],
                            prt[:, i * QW:i * QW + cw],
                            start=(kt == 0), stop=(kt == nsk - 1))

                pending = [None]

                def emit_attention(qb, fillers0=(), fillers1=()):
                    nsk = (qb + 1) * 4
                    for hp in range(2):
                        fillers = list(fillers0 if hp == 0 else fillers1)
                        rate = len(fillers) / max(1, nsk - 2)
                        # start with one filler's credit banked: the extra PE
                        # work at pair start covers the previous pair's
                        # PSUM-release latency before the first PV
                        credit = 1.0
                        po = psO.tile([128, 2 * QW], f32, tag="pvacc",
                                      name=f"po_{hp}_{qb}")
                        prts = {}
                        for kt in range(nsk):
                            c0 = max(0, kt * 128 - qb * QW)
                            cw = QW - c0
                            strip = psS.tile([128, 2 * QW], f32, tag="score",
                                             name=f"sc_{hp}_{qb}_{kt}")
                            nc.tensor.matmul(
                                strip[:, 0:cw],
                                kTt[hp][kt // 4][0:64,
                                                 (kt % 4) * 128:
                                                 (kt % 4) * 128 + 128],
                                qT[hp][qb][0:64, c0:QW],
                                start=True, stop=True,
                                tile_position=(0, 0))
                            nc.tensor.matmul(
                                strip[:, QW:QW + cw],
                                kTt[hp][kt // 4][64:128,
                                                 (kt % 4) * 128:
                                                 (kt % 4) * 128 + 128],
                                qT[hp][qb][64:128, c0:QW],
                                start=True, stop=True,
                                tile_position=(64, 0))
                            prt = wp.tile([128, 2 * QW], MMDT, tag="probs",
                                          name=f"pr_{hp}_{qb}_{kt}")
                            sview = strip[:].rearrange(
                                "p (s q) -> p s q", q=QW)[:, :, 0:cw]
                            pview = prt[:].rearrange(
                                "p (s q) -> p s q", q=QW)[:, :, 0:cw]
                            nc.scalar.activation(pview, sview, Exp,
                                                 scale=0.125)
                            if kt >= nsk - 4:
                                nc.gpsimd.affine_select(
                                    out=pview, in_=pview,
                                    pattern=[[0, 2], [1, cw]], base=0,
                                    channel_multiplier=-1,
                                    compare_op=mybir.AluOpType.is_ge,
                                    fill=0.0)
                            prts[kt] = (prt, c0, cw)
                            # the previous pair's deferred tail (its last PV
                            # + normalize) lands here, AFTER this pair's
                            # first two score groups are in the PE queue:
                            # the select-gated final PV no longer leaves the
                            # PE with nothing runnable ahead of it
                            if kt == 1 and pending[0] is not None:
                                pending[0]()
                                pending[0] = None
                            # fillers may read tiles written by the deferred
                            # finish above — only pop after it has run
                            if 1 <= kt < nsk - 2:
                                credit += rate
                                while credit >= 1.0 and fillers:
                                    credit -= 1.0
                                    fillers.pop(0)()
                            if kt > 0:
                                emit_pv(po, hp, qb, kt - 1, nsk, prts)
                        def finish(po=po, hp=hp, qb=qb, nsk=nsk,
                                   prts=prts, leftovers=list(fillers)):
                            emit_pv(po, hp, qb, nsk - 1, nsk, prts)
                            # PSUM-freeing copies first (DVE den row in
                            # parallel with the ACT PV-row copy), then the
                            # reciprocal + broadcast + normalize multiplies
                            den1 = sp.tile([1, 2 * QW], f32, tag="den1")
                            nc.vector.tensor_copy(den1[:], po[HD:HD + 1, :])
                            poc = sp.tile([128, 2 * QW], f32, tag="poc")
                            nc.scalar.activation(
                                poc[0:HD, :], po[0:HD, :], Copy)
                            rc1 = sp.tile([1, 2 * QW], f32, tag="rc1")
                            nc.vector.reciprocal_approx_fast(
                                out=rc1[:], in_=den1[:])
                            rcb = sp.tile([64, 2 * QW], f32, tag="rcb")
                            nc.gpsimd.partition_broadcast(rcb[:], rc1[:])
                            nc.vector.tensor_tensor(
                                out=attnT[hp][qb][0:64, :],
                                in0=poc[0:HD, 0:QW], in1=rcb[:, 0:QW],
                                op=mybir.AluOpType.mult)
                            nc.vector.tensor_tensor(
                                out=attnT[hp][qb][64:128, :],
                                in0=poc[0:HD, QW:2 * QW],
                                in1=rcb[:, QW:2 * QW],
                                op=mybir.AluOpType.mult)
                            # leftover fillers flush after the normalize so
                            # their DVE work cannot delay the PSUM release
                            for f in leftovers:
                                f()
                        fillers.clear()
                        pending[0] = finish

                # ---- main wavefront ------------------------------------
                def qk_units(sb, hps=(0, 1)):
                    fs = []
                    for hp in hps:
                        for w_src, dst in ((wq_s, qT), (wk_s, kTt)):
                            fs.append(lambda w=w_src, d=dst, h=hp, s=sb:
                                      emit_qk_proj(w, d, h, s))
                    return fs

                def v_units(sb):
                    return [lambda t=st: emit_v(t)
                            for st in range(sb * 4, sb * 4 + 4)]

                def out_units(sb, on_act=False):
                    return [lambda t=st, d=db: emit_out(t, d, on_act)
                            for st in range(sb * 4, sb * 4 + 4)
                            for db in range(2)]

                def interleave(a, b):
                    fs, a, b = [], list(a), list(b)
                    while a or b:
                        if a:
                            fs.append(a.pop(0))
                        if b:
                            fs.append(b.pop(0))
                    return fs

                # block 0 projections inline
                for f in interleave(qk_units(0), v_units(0)):
                    f()
                # trigger x1/wo loads only now, gated behind qT[0][0]: keeps
                # the first-needed loads alone on the DMA fabric early on
                gate = sp.tile([128, 1], MMDT, tag="gate")
                nc.scalar.activation(gate[:], qT[0][0][:, 0:1], Copy)
                load_x_block(1, eng=nc.scalar)
                nc.scalar.dma_start(wo_s[0][:], woT[0:128, :])
                nc.scalar.dma_start(wo_s[1][:], woT[128:256, :])
                load_x_block(2)
                # attn(0): fillers = projections of block 1
                emit_attention(0,
                               interleave(qk_units(1), v_units(1))[:4],
                               interleave(qk_units(1), v_units(1))[4:])
                load_x_block(3)
                # attn(1): projections of block 2 + out-proj of block 0 (ACT)
                p2 = interleave(qk_units(2), v_units(2))
                o0 = out_units(0)
                emit_attention(1, p2[:4] + o0[:2], p2[4:] + o0[2:])
                # attn(2): hp0 projections of block 3 + v3 + out-proj blk 1
                p3a = interleave(qk_units(3, hps=(0,)), v_units(3))
                o1 = out_units(1)
                emit_attention(2, p3a[:3] + o1[:4], p3a[3:] + o1[4:])
                # attn(3): pair0 gets hp1 projections of blk 3 + out blk 2;
                # pair1 gets the hp0 half of block 3's own out-projection
                o2 = out_units(2)
                hp0f = [lambda t=st, d=db: emit_out_hp0(t, d)
                        for st in range(12, 16) for db in range(2)]
                emit_attention(3, qk_units(3, hps=(1,)) + o2, hp0f)
                pending[0]()
                pending[0] = None
                for st in range(12, 16):
                    for db in range(2):
                        emit_out_hp1(st, db)

    nc.compile()
    return nc


def _rope_permute_cols(wT):
    """Permute the 64-dh blocks of the [D, GC] transposed weight so RoPE
    pairs (d, d+32) become adjacent columns (2i, 2i+1)."""
    w = wT.reshape(D, GH, HD).copy()
    perm = np.empty(HD, dtype=np.int64)
    perm[0::2] = np.arange(32)
    perm[1::2] = np.arange(32) + 32
    return np.ascontiguousarray(w[:, :, perm].reshape(D, GC))


def _shard_inputs(x, cos, sin, wq, wk, wv, wo):
    perm = np.empty(HD, dtype=np.int64)
    perm[0::2] = np.arange(32)
    perm[1::2] = np.arange(32) + 32
    cosP = np.asarray(cos, np.float32).reshape(S, HD)[:, perm].T  # [64, S]
    sinP = np.asarray(sin, np.float32).reshape(S, HD)[:, perm].T
    snF = sinP.copy()
    # snH = swap(sign-folded sin): the kernel computes swap(pq*snH), so the
    # -sin that lands on even output rows must sit on odd table rows.
    snF[1::2, :] *= -1.0
    cs2 = np.ascontiguousarray(
        np.concatenate([cosP, cosP], axis=0), dtype=np.float16)
    sn2 = np.ascontiguousarray(
        np.concatenate([snF, snF], axis=0), dtype=np.float16)
    x = np.asarray(x, np.float32)

    def to_sbuf_w(wT):
        # [D, GC] -> [128, KT*GC] in the on-chip layout (kt-major per line)
        return np.ascontiguousarray(
            wT.reshape(KT, 128, GC).transpose(1, 0, 2).reshape(128, KT * GC),
            dtype=np.float16)

    def to_sbuf_x(xTb):
        # [D, S] -> [128, QB*KT*QW]: per block cb, kt-major 512-col slices
        v = xTb.reshape(KT, 128, QB, QW).transpose(1, 2, 0, 3)
        return np.ascontiguousarray(
            v.reshape(128, QB * KT * QW), dtype=np.float16)

    in_maps = []
    for c in range(NCORES):
        b, g = c // GROUPS, c % GROUPS
        rows = slice(g * GC, (g + 1) * GC)
        wqTg = _rope_permute_cols(np.asarray(wq, np.float32)[rows, :].T)
        wkTg = _rope_permute_cols(np.asarray(wk, np.float32)[rows, :].T)
        in_maps.append({
            "xS": to_sbuf_x(x[b].T),
            "wqS": to_sbuf_w(wqTg),
            "wkS": to_sbuf_w(wkTg),
            "wvS": to_sbuf_w(np.asarray(wv, np.float32)[rows, :].T),
            "woT": np.ascontiguousarray(
                np.asarray(wo, np.float32)[:, rows].T, dtype=np.float16),
            "cs2": cs2,
            "sn2": sn2,
        })
    return in_maps


def _run(inputs, trace=False, trace_kwargs=None):
    if "nc" not in _cache:
        _cache["nc"] = _build()
    nc = _cache["nc"]
    in_maps = _shard_inputs(
        inputs["x"], inputs["cos"], inputs["sin"],
        inputs["wq"], inputs["wk"], inputs["wv"], inputs["wo"])
    res = run_bass_kernel_spmd(
        nc, in_maps, list(range(NCORES)), trace=trace,
        **(trace_kwargs or {}))
    full = np.zeros((B, S, D), dtype=np.float32)
    for c in range(NCORES):
        full[c // GROUPS] += res.results[c]["out"].astype(np.float32)
        full[c // GROUPS][3 * QW:] += \
            res.results[c]["out3b"].astype(np.float32)
    return full, res


def kernel(**inputs):
    full, _ = _run(inputs, trace=False)
    return full


# revision 28
# speedup vs baseline: 1.1570x; 1.0002x over previous
"""Trainium2 Bass kernel for nn_Attention_84473416778449.

Reference computation (B=2, S=2048, D=1024, H=16, HD=64, fp32):
    q/k/v = x @ w{q,k,v}.T ; RoPE(q, k) ; causal softmax attention ; out @ wo.T

Sharding: 8 cores = (batch 2) x (head-group 4). Each core computes 4 heads of
one batch end-to-end and a partial output projection over its 256 channels;
the host sums the 4 partials per batch.

Key design points (v3):
  - All device inputs fp16 (host pre-converts); output fp16, summed on host.
  - RoPE without a PE matmul: host permutes q/k output channels so RoPE
    pairs (d, d+32) sit adjacent; rotate_half becomes a DVE stream_shuffle
    (adjacent-pair swap within 32-blocks), signs folded into the sin table.
  - Scores for the two heads of a pair run concurrently on the PE via row
    tiling (tile_position (0,0) / (64,0)), writing halves of one
    [128, 1024] PSUM strip consumed by a single batched exp per kt.
  - Wavefront + fillers: projections of later blocks and output projections
    of earlier blocks are interleaved between attention matmul groups so no
    engine idles long; filler lists are balanced per phase.
  - Pair-boundary decoupling: at each pair end only the two PSUM-freeing
    copies run immediately (ACT: PV rows, DVE: denominator row); the
    reciprocal/broadcast/normalize-multiplies are deferred into the next
    pair's iterations.
  - Final block output projection is hp-split: the attnT[0] half runs as
    fillers during the last pair; only the attnT[1] matmul + add + DMA
    remain in the tail.
  - PSUM (8 banks): proj 2 + score strips 4 + PV accumulator pair 2.
"""
import sys

if "/opt/trn_rl_repo" not in sys.path:
    sys.path.insert(0, "/opt/trn_rl_repo")

import numpy as np

import concourse.bass as bass
import concourse.mybir as mybir
import concourse.tile as tile
from concourse import bacc
from concourse.bass_utils import run_bass_kernel_spmd

B, S, D, H, HD = 2, 2048, 1024, 16, 64
NCORES = 8
GROUPS = 4            # head groups
GH = H // GROUPS      # heads per group = 4
GC = GH * HD          # channels per group = 256
KT = D // 128         # 8 k-tiles over D
ST = S // 128         # 16 s-tiles
QB = 4                # sq blocks of 512
QW = S // QB          # 512
VW = GH * (HD + 1)    # 260: v tile payload columns
VPAD = 3 * (HD + 1) + 128   # 323: pad so stationary window fits for h=3

f32 = mybir.dt.float32
MMDT = mybir.dt.float16   # matmul-operand dtype
Exp = mybir.ActivationFunctionType.Exp
Copy = mybir.ActivationFunctionType.Copy

# stream_shuffle mask: swap adjacent pairs within each 32-partition quadrant
SWAP_MASK = [j ^ 1 for j in range(32)]

# stride-0 partition APs are rejected by the DVE ("partition dimension must
# have nonzero step") — go through the gpsimd partition_broadcast
USE_BCAST_AP = False

_cache = {}


def _build():
    nc = bacc.Bacc("TRN2", num_devices=NCORES)

    # x and q/k/v weights arrive pre-arranged in the exact SBUF layout so
    # every load is a contiguous full-rate DMA (8KB / 4KB partition lines)
    xS = nc.dram_tensor("xS", [128, QB * KT * QW], MMDT,
                        kind="ExternalInput").ap()
    wqS = nc.dram_tensor("wqS", [128, KT * GC], MMDT,
                         kind="ExternalInput").ap()
    wkS = nc.dram_tensor("wkS", [128, KT * GC], MMDT,
                         kind="ExternalInput").ap()
    wvS = nc.dram_tensor("wvS", [128, KT * GC], MMDT,
                         kind="ExternalInput").ap()
    woT = nc.dram_tensor("woT", [GC, D], MMDT, kind="ExternalInput").ap()
    cs2 = nc.dram_tensor("cs2", [128, S], MMDT, kind="ExternalInput").ap()
    sn2 = nc.dram_tensor("sn2", [128, S], MMDT, kind="ExternalInput").ap()
    out = nc.dram_tensor("out", [S, D], MMDT, kind="ExternalOutput").ap()
    # block-3 rows get an hp-split output: the wo[0:128] half lands in `out`,
    # the wo[128:256] half in `out3b`; the host sums them
    out3b = nc.dram_tensor("out3b", [QW, D], MMDT,
                           kind="ExternalOutput").ap()

    with tile.TileContext(nc) as tc:
        with tc.tile_pool(name="persist", bufs=1) as pp, \
             tc.tile_pool(name="rope", bufs=3) as rp, \
             tc.tile_pool(name="probs", bufs=4) as wp, \
             tc.tile_pool(name="outsb", bufs=3) as op_, \
             tc.tile_pool(name="norm", bufs=2) as sp:

            # ---- first-needed loads go out before anything else ------------
            xTb = [pp.tile([128, KT * QW], MMDT, tag=f"xTb{cb}",
                           name=f"xTb{cb}") for cb in range(QB)]

            def xTr(kt, cb):
                return xTb[cb][:, kt * QW:(kt + 1) * QW]

            def load_x_block(cb, chunks=1, eng=None):
                eng = eng or nc.gpsimd
                w = KT * QW // chunks
                for j in range(chunks):
                    eng.dma_start(
                        xTb[cb][:, j * w:(j + 1) * w],
                        xS[:, cb * KT * QW + j * w:cb * KT * QW + (j + 1) * w])

            def load_wT(srcS, eng):
                t = pp.tile([128, KT * GC], MMDT, tag=f"w{srcS.tensor.name}",
                            name=f"w{srcS.tensor.name}")
                eng.dma_start(t[:], srcS[:])
                return t

            # spread the startup loads over the three DMA-capable queues;
            # x1 and wo are triggered later, gated behind compute sems, so
            # the first-needed loads monopolize early DMA bandwidth
            wq_s = load_wT(wqS, nc.sync)
            load_x_block(0, chunks=4)
            wv_s = load_wT(wvS, nc.scalar)
            cs_sb = pp.tile([128, S], MMDT, tag="cs")
            nc.sync.dma_start(cs_sb[:], cs2[:])
            sn_sb = pp.tile([128, S], MMDT, tag="sn")
            nc.sync.dma_start(sn_sb[:], sn2[:])
            wk_s = load_wT(wkS, nc.scalar)
            wo_s = []
            for kt in range(2):
                t = pp.tile([128, D], MMDT, tag=f"wo{kt}", name=f"wo{kt}")
                wo_s.append(t)

            qT = [[pp.tile([128, QW], MMDT, tag=f"qT{i}_{b}",
                           name=f"qT{i}_{b}") for b in range(QB)]
                  for i in range(2)]
            kTt = [[pp.tile([128, QW], MMDT, tag=f"kT{i}_{b}",
                            name=f"kT{i}_{b}") for b in range(QB)]
                   for i in range(2)]
            attnT = [[pp.tile([128, QW], MMDT, tag=f"aT{i}_{b}",
                              name=f"aT{i}_{b}") for b in range(QB)]
                     for i in range(2)]
            v_sb = [pp.tile([128, VPAD], MMDT, tag=f"v{i}",
                            name=f"v{i}") for i in range(ST)]
            # one-time v-tile framing: ones column per head (the PV
            # denominator lands in psum row 64; engine partition bases must
            # be 32-aligned so row 64 is staged via a 1-partition DVE copy)
            for st in range(ST):
                vhe = v_sb[st][:, 0:VW].rearrange("p (h e) -> p h e", e=HD + 1)
                nc.vector.memset(vhe[:, :, HD:HD + 1], 1.0)
                nc.vector.memset(v_sb[st][:, VW:VPAD], 0.0)

            with tc.tile_pool(name="psP", bufs=2, space="PSUM") as psP, \
                 tc.tile_pool(name="psS", bufs=2, space="PSUM") as psS, \
                 tc.tile_pool(name="psO", bufs=1, space="PSUM") as psO:

                def emit_qk_proj(w_src, dst, hp, sb):
                    cols = slice(sb * QW, (sb + 1) * QW)
                    pq = psP.tile([128, QW], f32, tag="proj",
                                  name=f"pq_{0 if dst is qT else 1}_{hp}_{sb}")
                    for kt in range(KT):
                        nc.tensor.matmul(
                            pq[:],
                            w_src[:, kt * GC + hp * 128:
                                  kt * GC + hp * 128 + 128],
                            xTr(kt, sb),
                            start=(kt == 0), stop=(kt == KT - 1))
                    # RoPE: dst = pqh*cos + swap(pqh*snH); snH is the
                    # pre-swapped sign-folded sin table (host-built); swap is
                    # the adjacent-pair partition shuffle (fp16 in/out).
                    # pq is staged to fp16 SBUF first (DVE copy, 2x mode):
                    # this frees the PSUM bank quickly and lets the DVE
                    # multiplies run in 2x packed mode.
                    pqh = rp.tile([128, QW], MMDT, tag="pqh")
                    nc.vector.tensor_copy(pqh[:], pq[:])
                    tcs = rp.tile([128, QW], MMDT, tag="tcs")
                    nc.vector.tensor_tensor(
                        out=tcs[:], in0=pqh[:], in1=cs_sb[:, cols],
                        op=mybir.AluOpType.mult)
                    tsp = rp.tile([128, QW], MMDT, tag="tsp")
                    nc.vector.tensor_tensor(
                        out=tsp[:], in0=pqh[:], in1=sn_sb[:, cols],
                        op=mybir.AluOpType.mult)
                    tsn = rp.tile([128, QW], MMDT, tag="tsn")
                    nc.vector.stream_shuffle(tsn[:], tsp[:], SWAP_MASK)
                    nc.vector.tensor_tensor(
                        out=dst[hp][sb][:], in0=tcs[:], in1=tsn[:],
                        op=mybir.AluOpType.add)

                def emit_v(st):
                    pv = psP.tile([128, QW], f32, tag="proj",
                                  name=f"pv_{st}")
                    for kt in range(KT):
                        nc.tensor.matmul(
                            pv[:, 0:GC],
                            xTr(kt, st // 4)[:, (st % 4) * 128:
                                             (st % 4) * 128 + 128],
                            wv_s[:, kt * GC:(kt + 1) * GC],
                            start=(kt == 0), stop=(kt == KT - 1))
                    vhe = v_sb[st][:, 0:VW].rearrange(
                        "p (h e) -> p h e", e=HD + 1)
                    nc.vector.tensor_copy(
                        vhe[:, :, 0:HD],
                        pv[:, 0:GC].rearrange("p (h d) -> p h d", d=HD))

                def emit_out(st, db, on_act=False):
                    pc = psP.tile([128, QW], f32, tag="proj",
                                  name=f"pc_{st}_{db}")
                    for hp in range(2):
                        nc.tensor.matmul(
                            pc[:],
                            attnT[hp][st // 4][:, (st % 4) * 128:
                                               (st % 4) * 128 + 128],
                            wo_s[hp][:, db * QW:(db + 1) * QW],
                            start=(hp == 0), stop=(hp == 1))
                    ob = op_.tile([128, QW], MMDT, tag="outsb")
                    if on_act:
                        nc.scalar.activation(ob[:], pc[:], Copy)
                    else:
                        nc.vector.tensor_copy(ob[:], pc[:])
                    nc.sync.dma_start(
                        out[st * 128:(st + 1) * 128,
                            db * QW:(db + 1) * QW], ob[:])

                # hp-split output projection for the final block: each half
                # is DMA'd as its own partial (host sums), so the tail has no
                # serial DVE adds; the hp1 copies ride the idle ACT engine
                def emit_out_hp0(st, db):
                    pc = psP.tile([128, QW], f32, tag="proj",
                                  name=f"pc0_{st}_{db}")
                    nc.tensor.matmul(
                        pc[:],
                        attnT[0][st // 4][:, (st % 4) * 128:
                                          (st % 4) * 128 + 128],
                        wo_s[0][:, db * QW:(db + 1) * QW],
                        start=True, stop=True)
                    ob = op_.tile([128, QW], MMDT, tag="outsb")
                    nc.vector.tensor_copy(ob[:], pc[:])
                    nc.sync.dma_start(
                        out[st * 128:(st + 1) * 128,
                            db * QW:(db + 1) * QW], ob[:])

                def emit_out_hp1(st, db):
                    pc = psP.tile([128, QW], f32, tag="proj",
                                  name=f"pc1_{st}_{db}")
                    nc.tensor.matmul(
                        pc[:],
                        attnT[1][st // 4][:, (st % 4) * 128:
                                          (st % 4) * 128 + 128],
                        wo_s[1][:, db * QW:(db + 1) * QW],
                        start=True, stop=True)
                    ob = op_.tile([128, QW], MMDT, tag="outsb")
                    nc.scalar.activation(ob[:], pc[:], Copy)
                    nc.sync.dma_start(
                        out3b[(st - 12) * 128:(st - 11) * 128,
                              db * QW:(db + 1) * QW], ob[:])


                def emit_pv(po, hp, qb, kt, nsk, prts):
                    prt, c0, cw = prts.pop(kt)
                    for i, h in enumerate((2 * hp, 2 * hp + 1)):
                        nc.tensor.matmul(
                            po[:, i * QW + c0:(i + 1) * QW],
                            v_sb[kt][:, h * (HD + 1):h * (HD + 1) + 128],
                            prt[:, i * QW:i * QW + cw],
                            start=(kt == 0), stop=(kt == nsk - 1))

                pending = [None]

                def emit_attention(qb, fillers0=(), fillers1=()):
                    nsk = (qb + 1) * 4
                    for hp in range(2):
                        fillers = list(fillers0 if hp == 0 else fillers1)
                        rate = len(fillers) / max(1, nsk - 2)
                        # start with two fillers' credit banked: the extra
                        # PE work at pair start covers the previous pair's
                        # PSUM-release latency before the first PV
                        credit = 2.0
                        po = psO.tile([128, 2 * QW], f32, tag="pvacc",
                                      name=f"po_{hp}_{qb}")
                        prts = {}
                        for kt in range(nsk):
                            c0 = max(0, kt * 128 - qb * QW)
                            cw = QW - c0
                            strip = psS.tile([128, 2 * QW], f32, tag="score",
                                             name=f"sc_{hp}_{qb}_{kt}")
                            nc.tensor.matmul(
                                strip[:, 0:cw],
                                kTt[hp][kt // 4][0:64,
                                                 (kt % 4) * 128:
                                                 (kt % 4) * 128 + 128],
                                qT[hp][qb][0:64, c0:QW],
                                start=True, stop=True,
                                tile_position=(0, 0))
                            nc.tensor.matmul(
                                strip[:, QW:QW + cw],
                                kTt[hp][kt // 4][64:128,
                                                 (kt % 4) * 128:
                                                 (kt % 4) * 128 + 128],
                                qT[hp][qb][64:128, c0:QW],
                                start=True, stop=True,
                                tile_position=(64, 0))
                            prt = wp.tile([128, 2 * QW], MMDT, tag="probs",
                                          name=f"pr_{hp}_{qb}_{kt}")
                            sview = strip[:].rearrange(
                                "p (s q) -> p s q", q=QW)[:, :, 0:cw]
                            pview = prt[:].rearrange(
                                "p (s q) -> p s q", q=QW)[:, :, 0:cw]
                            nc.scalar.activation(pview, sview, Exp,
                                                 scale=0.125)
                            if kt >= nsk - 4:
                                nc.gpsimd.affine_select(
                                    out=pview, in_=pview,
                                    pattern=[[0, 2], [1, cw]], base=0,
                                    channel_multiplier=-1,
                                    compare_op=mybir.AluOpType.is_ge,
                                    fill=0.0)
                            prts[kt] = (prt, c0, cw)
                            # the previous pair's deferred tail (its last PV
                            # + normalize) lands here, AFTER this pair's
                            # first two score groups are in the PE queue:
                            # the select-gated final PV no longer leaves the
                            # PE with nothing runnable ahead of it
                            if kt == 1 and pending[0] is not None:
                                pending[0]()
                                pending[0] = None
                            # fillers may read tiles written by the deferred
                            # finish above — only pop after it has run
                            if 1 <= kt < nsk - 2:
                                credit += rate
                                while credit >= 1.0 and fillers:
                                    credit -= 1.0
                                    fillers.pop(0)()
                            if kt > 0:
                                emit_pv(po, hp, qb, kt - 1, nsk, prts)
                        def finish(po=po, hp=hp, qb=qb, nsk=nsk,
                                   prts=prts, leftovers=list(fillers)):
                            emit_pv(po, hp, qb, nsk - 1, nsk, prts)
                            # PSUM-freeing copies first (DVE den row in
                            # parallel with the ACT PV-row copy), then the
                            # reciprocal + broadcast + normalize multiplies
                            den1 = sp.tile([1, 2 * QW], f32, tag="den1")
                            nc.vector.tensor_copy(den1[:], po[HD:HD + 1, :])
                            poc = sp.tile([128, 2 * QW], f32, tag="poc")
                            nc.scalar.activation(
                                poc[0:HD, :], po[0:HD, :], Copy)
                            rc1 = sp.tile([1, 2 * QW], f32, tag="rc1")
                            nc.vector.reciprocal_approx_fast(
                                out=rc1[:], in_=den1[:])
                            rcb = sp.tile([64, 2 * QW], f32, tag="rcb")
                            nc.gpsimd.partition_broadcast(rcb[:], rc1[:])
                            nc.vector.tensor_tensor(
                                out=attnT[hp][qb][0:64, :],
                                in0=poc[0:HD, 0:QW], in1=rcb[:, 0:QW],
                                op=mybir.AluOpType.mult)
                            nc.vector.tensor_tensor(
                                out=attnT[hp][qb][64:128, :],
                                in0=poc[0:HD, QW:2 * QW],
                                in1=rcb[:, QW:2 * QW],
                                op=mybir.AluOpType.mult)
                            # leftover fillers flush after the normalize so
                            # their DVE work cannot delay the PSUM release
                            for f in leftovers:
                                f()
                        fillers.clear()
                        pending[0] = finish

                # ---- main wavefront ------------------------------------
                def qk_units(sb, hps=(0, 1)):
                    fs = []
                    for hp in hps:
                        for w_src, dst in ((wq_s, qT), (wk_s, kTt)):
                            fs.append(lambda w=w_src, d=dst, h=hp, s=sb:
                                      emit_qk_proj(w, d, h, s))
                    return fs

                def v_units(sb):
                    return [lambda t=st: emit_v(t)
                            for st in range(sb * 4, sb * 4 + 4)]

                def out_units(sb, on_act=False):
                    return [lambda t=st, d=db: emit_out(t, d, on_act)
                            for st in range(sb * 4, sb * 4 + 4)
                            for db in range(2)]

                def interleave(a, b):
                    fs, a, b = [], list(a), list(b)
                    while a or b:
                        if a:
                            fs.append(a.pop(0))
                        if b:
                            fs.append(b.pop(0))
                    return fs

                # block 0 projections inline
                for f in interleave(qk_units(0), v_units(0)):
                    f()
                # trigger x1/wo loads only now, gated behind qT[0][0]: keeps
                # the first-needed loads alone on the DMA fabric early on
                gate = sp.tile([128, 1], MMDT, tag="gate")
                nc.scalar.activation(gate[:], qT[0][0][:, 0:1], Copy)
                load_x_block(1, eng=nc.scalar)
                nc.scalar.dma_start(wo_s[0][:], woT[0:128, :])
                nc.scalar.dma_start(wo_s[1][:], woT[128:256, :])
                load_x_block(2)
                # attn(0): fillers = projections of block 1
                emit_attention(0,
                               interleave(qk_units(1), v_units(1))[:4],
                               interleave(qk_units(1), v_units(1))[4:])
                load_x_block(3)
                # attn(1): projections of block 2 + out-proj of block 0 (ACT)
                p2 = interleave(qk_units(2), v_units(2))
                o0 = out_units(0)
                emit_attention(1, p2[:4] + o0[:2], p2[4:] + o0[2:])
                # attn(2): hp0 projections of block 3 + v3 + out-proj blk 1
                p3a = interleave(qk_units(3, hps=(0,)), v_units(3))
                o1 = out_units(1)
                emit_attention(2, p3a[:3] + o1[:4], p3a[3:] + o1[4:])
                # attn(3): pair0 gets hp1 projections of blk 3 + out blk 2;
                # pair1 gets the hp0 half of block 3's own out-projection
                o2 = out_units(2)
                hp0f = [lambda t=st, d=db: emit_out_hp0(t, d)
                        for st in range(12, 16) for db in range(2)]
                emit_attention(3, qk_units(3, hps=(1,)) + o2, hp0f)
                pending[0]()
                pending[0] = None
                for st in range(12, 16):
                    for db in range(2):
                        emit_out_hp1(st, db)

    nc.compile()
    return nc


def _rope_permute_cols(wT):
    """Permute the 64-dh blocks of the [D, GC] transposed weight so RoPE
    pairs (d, d+32) become adjacent columns (2i, 2i+1)."""
    w = wT.reshape(D, GH, HD).copy()
    perm = np.empty(HD, dtype=np.int64)
    perm[0::2] = np.arange(32)
    perm[1::2] = np.arange(32) + 32
    return np.ascontiguousarray(w[:, :, perm].reshape(D, GC))


def _shard_inputs(x, cos, sin, wq, wk, wv, wo):
    perm = np.empty(HD, dtype=np.int64)
    perm[0::2] = np.arange(32)
    perm[1::2] = np.arange(32) + 32
    cosP = np.asarray(cos, np.float32).reshape(S, HD)[:, perm].T  # [64, S]
    sinP = np.asarray(sin, np.float32).reshape(S, HD)[:, perm].T
    snF = sinP.copy()
    # snH = swap(sign-folded sin): the kernel computes swap(pq*snH), so the
    # -sin that lands on even output rows must sit on odd table rows.
    snF[1::2, :] *= -1.0
    cs2 = np.ascontiguousarray(
        np.concatenate([cosP, cosP], axis=0), dtype=np.float16)
    sn2 = np.ascontiguousarray(
        np.concatenate([snF, snF], axis=0), dtype=np.float16)
    x = np.asarray(x, np.float32)

    def to_sbuf_w(wT):
        # [D, GC] -> [128, KT*GC] in the on-chip layout (kt-major per line)
        return np.ascontiguousarray(
            wT.reshape(KT, 128, GC).transpose(1, 0, 2).reshape(128, KT * GC),
            dtype=np.float16)

    def to_sbuf_x(xTb):
        # [D, S] -> [128, QB*KT*QW]: per block cb, kt-major 512-col slices
        v = xTb.reshape(KT, 128, QB, QW).transpose(1, 2, 0, 3)
        return np.ascontiguousarray(
            v.reshape(128, QB * KT * QW), dtype=np.float16)

    in_maps = []
    for c in range(NCORES):
        b, g = c // GROUPS, c % GROUPS
        rows = slice(g * GC, (g + 1) * GC)
        wqTg = _rope_permute_cols(np.asarray(wq, np.float32)[rows, :].T)
        wkTg = _rope_permute_cols(np.asarray(wk, np.float32)[rows, :].T)
        in_maps.append({
            "xS": to_sbuf_x(x[b].T),
            "wqS": to_sbuf_w(wqTg),
            "wkS": to_sbuf_w(wkTg),
            "wvS": to_sbuf_w(np.asarray(wv, np.float32)[rows, :].T),
            "woT": np.ascontiguousarray(
                np.asarray(wo, np.float32)[:, rows].T, dtype=np.float16),
            "cs2": cs2,
            "sn2": sn2,
        })
    return in_maps


def _run(inputs, trace=False, trace_kwargs=None):
    if "nc" not in _cache:
        _cache["nc"] = _build()
    nc = _cache["nc"]
    in_maps = _shard_inputs(
        inputs["x"], inputs["cos"], inputs["sin"],
        inputs["wq"], inputs["wk"], inputs["wv"], inputs["wo"])
    res = run_bass_kernel_spmd(
        nc, in_maps, list(range(NCORES)), trace=trace,
        **(trace_kwargs or {}))
    full = np.zeros((B, S, D), dtype=np.float32)
    for c in range(NCORES):
        full[c // GROUPS] += res.results[c]["out"].astype(np.float32)
        full[c // GROUPS][3 * QW:] += \
            res.results[c]["out3b"].astype(np.float32)
    return full, res


def kernel(**inputs):
    full, _ = _run(inputs, trace=False)
    return full
